# revision 14
# baseline (speedup 1.0000x reference)
"""Group-quantized linear (fake int4 per-group dequant) GEMV on 8 Trainium2 cores.

Reference computation (all fp32):
    qw = round_half_even(clip(W, -8, 7))            # W in [-8, 7) so clip is identity
    out = (qw.reshape(O, 64, 128) * scales[:, :, None]).reshape(O, O) @ x

Sharding: column-parallel — each core owns a 1024-row slice of W/scales,
x replicated, outputs concatenated (per the tensor-parallel hint).  The
per-core weight slice is shipped to the device pre-packed (pure host-side
layout): split into chunks of CHUNK_PLAN groups, each chunk laid out
[128 c-partitions, groups, 1024 o] so every SBUF partition's chunk data is
one contiguous HBM block, and each chunk is split into two halves packed
into two disjoint linear HBM regions — one per hardware DMA queue.

Per-core pipeline (device):
  DMA   : TWO HW DGE queues (SP + Activation engines) stream weight halves
          concurrently (~400 GB/s aggregate vs ~310 single-queue)
  DVE   : quantize via the fp32 magic-number trick (w + 1.5*2^23) - 1.5*2^23
          == round-half-even exactly for |w| < 2^22, cast to bf16 (exact for
          ints in [-8, 7]); single tensor_scalar op per chunk
  PE    : per (group g, out-chunk oc) matmul acc[oc][:, g, :2] =
          qwT[128c, 128o].T @ x2[128c, 2] where x2 = [x_hi | x_lo] bf16
          Dekker split of x (fp32-accurate), accumulated in fp32 PSUM
  DVE   : epilogue per oc: ONE fused tensor_tensor_reduce
          out[o] = sum_{g,j} acc[o, g, j] * scales[o, g]  (scales broadcast
          over the hi/lo axis via a stride-0 AP)
  PE/DVE: transpose [128, 8] result for a contiguous output DMA

Head: x DMA first on the SP queue (tiny), weights immediately after;
scales ride at the END of the Activation queue (needed only at epilogue).
Tail: CHUNK_PLAN ends in 4/2/1/1-group chunks so almost no work remains
after the last weight byte lands.
"""

import numpy as np

IN_DIM = 8192
OUT_DIM = 8192
NUM_GROUPS = 64
GROUP_SIZE = 128  # IN_DIM // NUM_GROUPS
N_CORES = 8
PER_OUT = OUT_DIM // N_CORES  # 1024
P = 128

MAGIC = np.float32(12582912.0)  # 1.5 * 2**23: (w + MAGIC) - MAGIC == rint(w)

# chunk sizes in groups; big chunks amortize DMA/DVE overhead, small tail
# chunks shrink the work left after the final weight byte arrives.
CHUNK_PLAN = [8] * 7 + [4, 2, 1, 1]
HALF_ELEMS = IN_DIM * PER_OUT // 2  # per-queue region size (f32 elems)

_cache = {}


def _chunk_halves():
    """(group_start, n_groups, a_elems) per chunk; a_elems == elems of the
    queue-A half (group-split for multi-group chunks, o-split for 1-group)."""
    out = []
    gs = 0
    for g in CHUNK_PLAN:
        if g >= 2:
            a = P * (g // 2) * PER_OUT
        else:
            a = P * (PER_OUT // 2)
        out.append((gs, g, a))
        gs += g
    return out


def _split_multi_waits(nc):
    """walrus in this container accepts only ONE sync-wait per instruction;
    Tile's tail drain carries one per producer proc. Hoist extras onto
    same-engine NoOps placed immediately before — identical semantics for an
    in-order sequencer."""
    import concourse.mybir as mybir

    uid = 0
    for f in nc.m.functions:
        for blk in f.blocks:
            insts = blk.instructions
            if not any(
                i.sync_info is not None
                and i.sync_info.on_wait
                and len(i.sync_info.on_wait) > 1
                for i in insts
            ):
                continue
            new_insts = []
            for inst in insts:
                si = inst.sync_info
                if si is not None and si.on_wait and len(si.on_wait) > 1:
                    waits = list(si.on_wait)
                    for w in waits[:-1]:
                        uid += 1
                        new_insts.append(
                            mybir.InstNoOp(
                                name=f"I-waitsplit-{uid}",
                                engine=inst.engine,
                                ins=[],
                                outs=[],
                                sync_info=mybir.SyncInfo(on_wait=[w], on_update=[]),
                            )
                        )
                    inst.sync_info = mybir.SyncInfo(
                        on_wait=[waits[-1]], on_update=si.on_update
                    )
                new_insts.append(inst)
            blk.instructions = new_insts
    return nc


def build_nc(w_bufs=3, split_waits=True):
    import concourse.bass as bass
    import concourse.mybir as mybir
    import concourse.tile as tile
    from concourse.masks import make_identity

    f32 = mybir.dt.float32
    bf16 = mybir.dt.bfloat16
    add = mybir.AluOpType.add

    ng = NUM_GROUPS
    oc_n = PER_OUT // P  # out-chunks of 128
    gpc_max = max(CHUNK_PLAN)

    nc = bass.Bass()
    # two per-queue linear regions, each HALF_ELEMS f32
    wa_d = nc.dram_tensor("wa", [HALF_ELEMS], f32, kind="ExternalInput")
    wb_d = nc.dram_tensor("wb", [HALF_ELEMS], f32, kind="ExternalInput")
    x_d = nc.dram_tensor("x", [IN_DIM], f32, kind="ExternalInput")
    sc_d = nc.dram_tensor("scales", [P, oc_n, ng], f32, kind="ExternalInput")
    out_d = nc.dram_tensor("out", [PER_OUT], f32, kind="ExternalOutput")

    with tile.TileContext(nc) as tc:
        with (
            tc.tile_pool(name="singles", bufs=1) as singles,
            tc.tile_pool(name="w", bufs=w_bufs) as wpool,
            tc.tile_pool(name="q", bufs=2) as qpool,
            tc.tile_pool(name="ep", bufs=2) as epool,
            tc.tile_pool(name="psum", bufs=1, space="PSUM") as psum,
        ):
            # ---- x load first on the SP queue (tiny), then weights flow.
            x_nat = singles.tile([ng, GROUP_SIZE], f32)
            nc.sync.dma_start(x_nat, x_d.rearrange("(g c) -> g c", c=GROUP_SIZE))

            # ---- weight stream: chunk halves A on SP queue, B on Act queue
            halves = _chunk_halves()
            wtiles = []
            sc_sb = singles.tile([P, oc_n, ng], f32)
            offa = offb = 0
            for ci, (gs, g, a_elems) in enumerate(halves):
                if ci == len(halves) - 2:
                    # scales ride late on the Act queue — needed only at the
                    # epilogue, but with enough lead to prep sc2 off the tail
                    nc.scalar.dma_start(sc_sb, sc_d[:])
                wf = wpool.tile([P, gpc_max, PER_OUT], f32, tag="wf", name=f"wf{ci}")
                b_elems = P * g * PER_OUT - a_elems
                if g >= 2:
                    h = g // 2
                    nc.sync.dma_start(
                        wf[:, :h, :],
                        wa_d[offa : offa + a_elems].rearrange(
                            "(c g o) -> c g o", c=P, g=h
                        ),
                    )
                    nc.scalar.dma_start(
                        wf[:, h : g, :],
                        wb_d[offb : offb + b_elems].rearrange(
                            "(c g o) -> c g o", c=P, g=g - h
                        ),
                    )
                else:
                    ho = PER_OUT // 2
                    nc.sync.dma_start(
                        wf[:, 0:1, :ho],
                        wa_d[offa : offa + a_elems].rearrange(
                            "(c g o) -> c g o", c=P, g=1
                        ),
                    )
                    nc.scalar.dma_start(
                        wf[:, 0:1, ho:],
                        wb_d[offb : offb + b_elems].rearrange(
                            "(c g o) -> c g o", c=P, g=1
                        ),
                    )
                offa += a_elems
                offb += b_elems
                wtiles.append(wf)



            # ---- x prep: PE-transpose [ng,128] -> [128,ng], Dekker-split
            # into interleaved bf16 hi/lo [128, ng, 2].
            ident_g = singles.tile([ng, ng], f32)
            make_identity(nc, ident_g)
            ident_p = singles.tile([P, P], f32)
            make_identity(nc, ident_p)

            x_ps = psum.tile([P, ng], f32, tag="paux")
            nc.tensor.transpose(x_ps, x_nat, ident_g)
            xT = singles.tile([P, ng], f32)
            nc.vector.tensor_copy(out=xT, in_=x_ps)
            xhi = singles.tile([P, ng], bf16)
            nc.vector.tensor_copy(out=xhi, in_=xT)
            xhi32 = singles.tile([P, ng], f32)
            nc.vector.tensor_copy(out=xhi32, in_=xhi)
            xlo32 = singles.tile([P, ng], f32)
            nc.vector.tensor_tensor(xlo32, xT, xhi32, mybir.AluOpType.subtract)
            x2 = singles.tile([P, ng, 2], bf16)
            nc.vector.tensor_copy(out=x2[:, :, 0], in_=xhi)
            nc.vector.tensor_copy(out=x2[:, :, 1], in_=xlo32)

            # persistent per-out-chunk PSUM accumulators [128, ng, 2]
            # tag paux is shared with x_ps (released above) and the final
            # output-transpose tile, keeping total PSUM slots == oc_n + 1.
            acc = [
                psum.tile(
                    [P, ng, 2],
                    f32,
                    tag=f"pacc{i}" if i else "paux",
                    name=f"acc{i}",
                )
                for i in range(oc_n)
            ]

            # ---- main loop: quantize each chunk on arrival, gemv per group
            for ci, (gs, g, _) in enumerate(halves):
                wf = wtiles[ci]
                qw = qpool.tile(
                    [P, gpc_max, PER_OUT], bf16, tag="qw", name=f"qw{ci}"
                )
                nc.vector.tensor_scalar(
                    out=qw[:, :g, :],
                    in0=wf[:, :g, :],
                    scalar1=float(MAGIC),
                    scalar2=-float(MAGIC),
                    op0=add,
                    op1=add,
                )
                for gp in range(g):
                    for oc in range(oc_n):
                        nc.tensor.matmul(
                            acc[oc][:, gs + gp, :],
                            lhsT=qw[:, gp, oc * P : (oc + 1) * P],
                            rhs=x2[:, gs + gp, :],
                            start=True,
                            stop=True,
                        )

            # ---- epilogue: out[o] = sum_{g,j} acc[o,g,j] * scales[o,g]
            # two DVE ops per oc (tensor_tensor_reduce doesn't compile on
            # this walrus): multiply the flat [128, 128] PSUM block by
            # hi/lo-duplicated scales, then reduce — the hi+lo combine
            # happens inside the reduction for free.  sc2 is prepped right
            # after the last quantize so it overlaps the final matmuls.
            sc2 = singles.tile([P, oc_n, ng, 2], f32)
            nc.vector.tensor_copy(out=sc2[:, :, :, 0], in_=sc_sb)
            nc.vector.tensor_copy(out=sc2[:, :, :, 1], in_=sc_sb)
            sc2f = sc2.rearrange("p oc g j -> p (oc g j)")
            out_sb = singles.tile([P, oc_n], f32)
            for oc in range(oc_n):
                ys = epool.tile([P, ng * 2], f32, tag="ys")
                nc.vector.tensor_tensor(
                    ys,
                    acc[oc].rearrange("p g j -> p (g j)"),
                    sc2f[:, oc * ng * 2 : (oc + 1) * ng * 2],
                    mybir.AluOpType.mult,
                )
                nc.vector.reduce_sum(
                    out=out_sb[:, oc : oc + 1],
                    in_=ys,
                    axis=mybir.AxisListType.X,
                )

            # ---- transpose [128, oc_n] -> [oc_n, 128] for a contiguous store
            o_ps = psum.tile([oc_n, P], f32, tag="paux")
            nc.tensor.transpose(o_ps, out_sb, ident_p)
            outT = singles.tile([oc_n, P], f32)
            nc.vector.tensor_copy(out=outT, in_=o_ps)
            nc.sync.dma_start(out_d.rearrange("(oc p) -> oc p", p=P), outT)

    return _split_multi_waits(nc) if split_waits else nc


def make_in_maps(x, weights, scales):
    """Per-core input staging (host-side layout only)."""
    x = np.ascontiguousarray(np.asarray(x, dtype=np.float32))
    weights = np.asarray(weights, dtype=np.float32)
    scales = np.asarray(scales, dtype=np.float32)
    oc_n = PER_OUT // P
    in_maps = []
    for c in range(N_CORES):
        sl = slice(c * PER_OUT, (c + 1) * PER_OUT)
        wtc = weights[sl].T  # [in_dim, per_out]
        parts_a, parts_b = [], []
        for gs, g, _ in _chunk_halves():
            # [128 c, g, 1024 o]: partition-contiguous chunk block
            blk = wtc[gs * P : (gs + g) * P, :].reshape(g, P, PER_OUT)
            blk = blk.transpose(1, 0, 2)
            if g >= 2:
                h = g // 2
                parts_a.append(blk[:, :h, :].ravel())
                parts_b.append(blk[:, h:, :].ravel())
            else:
                ho = PER_OUT // 2
                parts_a.append(blk[:, :, :ho].ravel())
                parts_b.append(blk[:, :, ho:].ravel())
        wa = np.ascontiguousarray(np.concatenate(parts_a))
        wb = np.ascontiguousarray(np.concatenate(parts_b))
        scc = np.ascontiguousarray(
            scales[sl].reshape(oc_n, P, NUM_GROUPS).transpose(1, 0, 2)
        )
        in_maps.append({"wa": wa, "wb": wb, "x": x, "scales": scc})
    return in_maps


def kernel(x, weights, scales):
    from concourse import bass_utils

    if "nc" not in _cache:
        _cache["nc"] = build_nc()
    nc = _cache["nc"]

    in_maps = make_in_maps(x, weights, scales)
    res = bass_utils.run_bass_kernel_spmd(nc, in_maps, core_ids=list(range(N_CORES)))
    return np.concatenate([res.results[c]["out"] for c in range(N_CORES)]).astype(
        np.float32
    )
